# revision 15
# baseline (speedup 1.0000x reference)
"""Trainium2 Bass kernel for the ConvolutionalKAN problem.

Math: the KAN conv
    out[b,o,y,x] = sum_{j,kk,l,m} phi_m(11*inp[b,j,y+kk,x+l]) * coeff[o,j,kk,l,m]
with phi_m the degree-3 B-spline basis on uniform knots linspace(0,1,12).
Uniform knots -> phi_m(t) = N3(t-m) with N3 the cardinal cubic B-spline:
    6*N3 = a^3 - 4*b^3,  a = relu(2-u), b = relu(1-u) = relu(a-1), u = |t-(m+2)|
Weights fold to coeff/6 exactly, making this a VALID 3x3 conv over
64*8 = 512 input channels.

v2 design (vs the 217us baseline, which was elementwise-bound):
- The whole two-rail spline evaluation collapses to 2 Scalar ACTs
  (u = Abs(11x - (m+2)), a = Relu(2-u)) plus ONE custom DVE op
  KAN_CUBE: g = a^3 - 4*relu(a-1)^3 (exactly 8 ALU stages), registered
  at runtime in dve_ops.OPS. This removes ~350us of engine time.
- Basis tiles g are written in bf16 (halves SBUF + matmul streams at the
  same 1 col/cycle as f32r); weights cast to bf16.
- Matmuls run tap-major: for each (chunk q, tap) the stationary lhsT is
  loaded once and 8 interleaved psum chains (one per 8-row output group)
  stream against it, keeping the PE continuously busy so it ramps to the
  full 2.4 GHz p-state. Each group accumulates 36 matmuls (4 chunks x 9
  taps) into one PSUM bank; all 8 banks are in flight.
- Images are processed in row-halves (0..33 / 32..63) so the PE starts
  ~6us after kernel start instead of waiting for a full-image basis.

Sharding: data-parallel over batch, 2 images per core on 8 cores.
"""

import os
import sys

import numpy as np

for _p in ("/root/.axon_site/_ro/trn_rl_repo", "/opt/trn_rl_repo"):
    if os.path.isdir(_p) and _p not in sys.path:
        sys.path.append(_p)

B_FULL = 16
N_CORES = 8
B_SHARD = B_FULL // N_CORES
CIN = 64
COUT = 64
H = 64
W = 64
KS = 3
NB = 8
NS = 8
HO = H - KS + 1  # 62
WO = W - KS + 1  # 62
NQ = (CIN * NS) // 128  # 4 contraction tiles of 128
TAPS = KS * KS
N_STEPS = NQ * TAPS  # 36 accumulation steps per psum chain

# row halves: (first input row, n input rows); groups 0-3 read rows 0..33,
# groups 4-7 read rows 32..63
HALVES = [(0, 34), (32, 32)]
# output row groups: 8 groups of 8 rows (last has 6): group g = out rows
# 8g .. 8g+nr-1, reading input rows 8g .. 8g+nr+1
GROUPS = [(g, 8 if g < 7 else 6) for g in range(8)]

MM_DTYPE_ENV = os.environ.get("KAN_MM_DTYPE", "bfloat16")

_DVE_OP_CACHE = {}


def _register_dve_op(name, spec):
    from concourse import dve_ops
    from concourse.dve_spec import lower
    from concourse.dve_uop import DveOpSpec
    from concourse.dve_spec import _has_src1

    existing = {op.name for op in dve_ops.OPS}
    if name in existing:
        return next(o for o in dve_ops.OPS if o.name == name)
    row = dve_ops._CUSTOM_DVE_ROW_BASE + len(dve_ops.OPS)
    shas = {}
    for ver in ("v3", "v4"):
        s = DveOpSpec(name=name, opcode=row, uops=lower(spec, ver=ver),
                      rd1_en=_has_src1(spec))
        shas[ver] = s.sha(ver)
    op = dve_ops.DveOp(name, spec, subdim=False, uops_sha=shas)
    dve_ops.OPS.append(op)
    dve_ops._SUB_OPCODE_FOR_NAME[name] = row
    return op


def _get_kan_ops():
    """Register (once) and return the two custom DVE ops:
    KAN_WIN:  a = relu(min(s0 - 11*x, 11*x - s1))  (= relu(2-|11x-(m+2)|)
              for s0 = m+4, s1 = m)
    KAN_CUBE: g = a^3 - 4*relu(a-1)^3              (= 6*N3(|11x-(m+2)|))
    """
    if "ops" in _DVE_OP_CACHE:
        return _DVE_OP_CACHE["ops"]
    from concourse.dve_spec import C0, C1, C2, One, Spec, Src0, minn, relu, sq

    m = Src0 * C2
    win_spec = Spec(
        body=relu(minn(C0 - m, m - C1)),
        reference=lambda in0, in1, s0, s1, imm2: np.maximum(
            np.minimum(s0 - in0 * imm2, in0 * imm2 - s1), 0.0
        ).astype(np.float32),
    )
    a = Src0
    b = relu(a - One)
    cube_spec = Spec(
        body=sq(a) * a + sq(b) * b * C2,
        reference=lambda in0, in1, s0, s1, imm2: (
            in0**3 + np.maximum(in0 - 1.0, 0.0) ** 3 * imm2
        ).astype(np.float32),
    )
    ops = (_register_dve_op("KAN_WIN_V1", win_spec),
           _register_dve_op("KAN_CUBE_V1", cube_spec))
    _DVE_OP_CACHE["ops"] = ops
    return ops


def _fold_coeff(coeff: np.ndarray):
    """coeff [COUT, CIN, KS, KS, NB] -> W_host [512, 576] f32.

    Channels fed to the matmul are 6*phi_m(t), so the folded weights are
    coeff/6 in layout W_host[m*64 + j, (kk*3+l)*64 + o].
    """
    w = (coeff.astype(np.float64).transpose(4, 1, 2, 3, 0) / 6.0).reshape(
        NS * CIN, TAPS * COUT)
    return np.ascontiguousarray(w, dtype=np.float32)


def _build_bass():
    import concourse.bacc as bacc
    import concourse.mybir as mybir
    import concourse.tile as tile

    f32 = mybir.dt.float32
    mm_dt = getattr(mybir.dt, MM_DTYPE_ENV)
    AF = mybir.ActivationFunctionType
    kan_win, kan_cube = _get_kan_ops()

    nc = bacc.Bacc("TRN2", target_bir_lowering=False, debug=False,
                   num_devices=N_CORES)
    x_d = nc.dram_tensor("x", [B_SHARD, CIN, H, W], f32, kind="ExternalInput").ap()
    w_d = nc.dram_tensor("w", [NS * CIN, TAPS * COUT], f32, kind="ExternalInput").ap()
    b_d = nc.dram_tensor("btbl", [128, 2 * NQ], f32, kind="ExternalInput").ap()
    out_d = nc.dram_tensor("out", [B_SHARD, COUT, HO, WO], f32,
                           kind="ExternalOutput").ap()

    with tile.TileContext(nc) as tc:
        from contextlib import ExitStack

        with ExitStack() as ctx:
            wpool = ctx.enter_context(tc.tile_pool(name="w", bufs=NQ))
            cpool = ctx.enter_context(tc.tile_pool(name="const", bufs=1))
            xpool = ctx.enter_context(tc.tile_pool(name="x", bufs=2))
            rpool = ctx.enter_context(tc.tile_pool(name="r", bufs=2))
            gpool = ctx.enter_context(tc.tile_pool(name="g", bufs=2))
            opool = ctx.enter_context(tc.tile_pool(name="o", bufs=2))
            ppool = ctx.enter_context(
                tc.tile_pool(name="ps", bufs=1, space="PSUM"))

            bt = cpool.tile([128, 2 * NQ], f32)
            nc.sync.dma_start(bt[:], b_d[:])
            wts = []
            for q in range(NQ):
                wt = wpool.tile([128, TAPS * COUT], f32, tag="wstage")
                nc.sync.dma_start(wt[:], w_d[q * 128:(q + 1) * 128, :])
                if mm_dt != f32:
                    wr = wpool.tile([128, TAPS * COUT], mm_dt, tag="wr")
                    nc.vector.tensor_copy(wr[:], wt[:])
                    wts.append(wr)
                else:
                    wts.append(wt)

            for b in range(B_SHARD):
                # --- basis: q-outer so matmuls can start after chunk 0 ---
                xts = []
                for h, (y0, nin) in enumerate(HALVES):
                    xt = xpool.tile([128, nin * W], f32, tag=f"xt{h}",
                                    name=f"xt{h}")
                    src = x_d[b, :, y0:y0 + nin, :]
                    xv = xt[:].rearrange("p (r c) -> p r c", c=W)
                    nc.gpsimd.dma_start(xv[0:64], src)
                    nc.gpsimd.dma_start(xv[64:128], src)
                    xts.append(xt)
                gts = {}  # (q, h) -> g tile [128, nrows*W] mm_dt
                for q in range(NQ):
                    for h, (y0, nin) in enumerate(HALVES):
                        npx = nin * W
                        a = rpool.tile([128, npx], f32, tag=f"a{h}",
                                       name=f"a{h}")
                        g = gpool.tile([128, npx], mm_dt, tag=f"g{q}{h}",
                                       name=f"g{q}{h}")
                        nc.vector._custom_dve(
                            kan_win, out=a[:], in0=xts[h][:],
                            s0=bt[:, 2 * q:2 * q + 1],
                            s1=bt[:, 2 * q + 1:2 * q + 2], imm2=11.0)
                        nc.vector._custom_dve(kan_cube, out=g[:], in0=a[:],
                                              imm2=-4.0)
                        gts[(q, h)] = g

                gvs = {k: g[:].rearrange("p (r c) -> p r c", c=W)
                       for k, g in gts.items()}

                # --- matmuls: tap-major, 8 interleaved psum chains ---
                pss = [ppool.tile([64, 8, WO], f32, tag=f"ps{g}",
                                  name=f"ps{g}")
                       for g, _ in GROUPS]
                step = 0
                for q in range(NQ):
                    for kk in range(KS):
                        for l in range(KS):
                            lhsT = wts[q][:, (kk * KS + l) * COUT:
                                          (kk * KS + l + 1) * COUT]
                            for g, nr in GROUPS:
                                h = g // 4
                                y0 = HALVES[h][0]
                                r0 = 8 * g + kk - y0
                                rhs = gvs[(q, h)][:, r0:r0 + nr, l:l + WO]
                                nc.tensor.matmul(
                                    pss[g][:, :nr, :], lhsT, rhs,
                                    start=(step == 0),
                                    stop=(step == N_STEPS - 1),
                                )
                            step += 1

                # --- drain: psum -> sbuf -> dram (split across S and V) ---
                for g, nr in GROUPS:
                    ot = opool.tile([64, 8, WO], f32, tag=f"ot{g % 4}",
                                    name=f"ot{g % 4}")
                    if g % 2 == 0:
                        nc.scalar.copy(ot[:, :nr, :], pss[g][:, :nr, :])
                    else:
                        nc.vector.tensor_copy(ot[:, :nr, :], pss[g][:, :nr, :])
                    nc.sync.dma_start(
                        out_d[b, :, 8 * g:8 * g + nr, :], ot[:, :nr, :])

    nc.compile()
    return nc


def _maybe_install_profile_shim():
    """Allow trace=True/BASS_TRACE under axon even though this image lacks
    antenv.axon_hooks; degrade silently if anything is missing."""
    import types

    if "antenv.axon_hooks" in sys.modules:
        return
    try:
        from trn_agent_boot.trn_boot import _ntff_profile_via_ctypes

        hook = _ntff_profile_via_ctypes("/opt/axon/libaxon_pjrt.so")
        if hook is None:
            return
        mod = types.ModuleType("antenv.axon_hooks")
        mod.get_axon_ntff_profile_hook = lambda: hook
        mod.set_axon_ntff_profile_hook = lambda h: None
        sys.modules["antenv.axon_hooks"] = mod
        from concourse import bass_utils

        bass_utils.upload_artifacts = lambda tmpdir: f"local:{tmpdir}"
    except Exception:
        pass


_LAST_RESULTS = None


def kernel(x: np.ndarray, coeff: np.ndarray) -> np.ndarray:
    global _LAST_RESULTS
    from concourse import bass_utils

    _maybe_install_profile_shim()

    x = np.ascontiguousarray(np.asarray(x), dtype=np.float32)
    coeff = np.asarray(coeff)
    assert x.shape == (B_FULL, CIN, H, W), x.shape

    w_host = _fold_coeff(coeff)
    btbl = np.zeros((128, 2 * NQ), dtype=np.float32)
    for p in range(128):
        for q in range(NQ):
            m = 2 * q + (1 if p >= 64 else 0)
            btbl[p, 2 * q] = float(m + 4)      # s0: a = relu(min(s0-11x, 11x-s1))
            btbl[p, 2 * q + 1] = float(m)      # s1

    nc = _build_bass()

    in_maps = []
    for i in range(N_CORES):
        in_maps.append({
            "x": np.ascontiguousarray(x[i * B_SHARD:(i + 1) * B_SHARD]),
            "w": w_host,
            "btbl": btbl,
        })

    res = bass_utils.run_bass_kernel_spmd(
        nc, in_maps, core_ids=list(range(N_CORES)),
        trace=bool(os.environ.get("KAN_TRACE")),
    )
    _LAST_RESULTS = res

    out = np.concatenate([res.results[i]["out"] for i in range(N_CORES)], axis=0)
    return out.astype(np.float32, copy=False)


# revision 20
# speedup vs baseline: 1.1068x; 1.1068x over previous
"""Trainium2 Bass kernel for the ConvolutionalKAN problem.

Math: the KAN conv
    out[b,o,y,x] = sum_{j,kk,l,m} phi_m(11*inp[b,j,y+kk,x+l]) * coeff[o,j,kk,l,m]
with phi_m the degree-3 B-spline basis on uniform knots linspace(0,1,12).
Uniform knots -> phi_m(t) = N3(t-m) with N3 the cardinal cubic B-spline:
    6*N3 = a^3 - 4*b^3,  a = relu(2-u), b = relu(1-u) = relu(a-1), u = |t-(m+2)|
Weights fold to coeff/6 exactly, making this a VALID 3x3 conv over
64*8 = 512 input channels.

v2 design (vs the 217us baseline, which was elementwise-bound):
- The whole two-rail spline evaluation collapses to 2 Scalar ACTs
  (u = Abs(11x - (m+2)), a = Relu(2-u)) plus ONE custom DVE op
  KAN_CUBE: g = a^3 - 4*relu(a-1)^3 (exactly 8 ALU stages), registered
  at runtime in dve_ops.OPS. This removes ~350us of engine time.
- Basis tiles g are written in bf16 (halves SBUF + matmul streams at the
  same 1 col/cycle as f32r); weights cast to bf16.
- Matmuls run tap-major: for each (chunk q, tap) the stationary lhsT is
  loaded once and 8 interleaved psum chains (one per 8-row output group)
  stream against it, keeping the PE continuously busy so it ramps to the
  full 2.4 GHz p-state. Each group accumulates 36 matmuls (4 chunks x 9
  taps) into one PSUM bank; all 8 banks are in flight.
- Images are processed in row-halves (0..33 / 32..63) so the PE starts
  ~6us after kernel start instead of waiting for a full-image basis.

Sharding: data-parallel over batch, 2 images per core on 8 cores.
"""

import os
import sys

import numpy as np

for _p in ("/root/.axon_site/_ro/trn_rl_repo", "/opt/trn_rl_repo"):
    if os.path.isdir(_p) and _p not in sys.path:
        sys.path.append(_p)

B_FULL = 16
N_CORES = 8
B_SHARD = B_FULL // N_CORES
CIN = 64
COUT = 64
H = 64
W = 64
KS = 3
NB = 8
NS = 8
HO = H - KS + 1  # 62
WO = W - KS + 1  # 62
NQ = (CIN * NS) // 128  # 4 contraction tiles of 128
TAPS = KS * KS
N_STEPS = NQ * TAPS  # 36 accumulation steps per psum chain

# row halves: (first input row, n input rows); groups 0-3 read rows 0..33,
# groups 4-7 read rows 32..63
HALVES = [(0, 34), (32, 32)]
# output row groups: 8 groups of 8 rows (last has 6): group g = out rows
# 8g .. 8g+nr-1, reading input rows 8g .. 8g+nr+1
GROUPS = [(g, 8 if g < 7 else 6) for g in range(8)]

MM_DTYPE_ENV = os.environ.get("KAN_MM_DTYPE", "bfloat16")

_DVE_OP_CACHE = {}


def _register_dve_op(name, spec):
    from concourse import dve_ops
    from concourse.dve_spec import lower
    from concourse.dve_uop import DveOpSpec
    from concourse.dve_spec import _has_src1

    existing = {op.name for op in dve_ops.OPS}
    if name in existing:
        return next(o for o in dve_ops.OPS if o.name == name)
    row = dve_ops._CUSTOM_DVE_ROW_BASE + len(dve_ops.OPS)
    shas = {}
    for ver in ("v3", "v4"):
        s = DveOpSpec(name=name, opcode=row, uops=lower(spec, ver=ver),
                      rd1_en=_has_src1(spec))
        shas[ver] = s.sha(ver)
    op = dve_ops.DveOp(name, spec, subdim=False, uops_sha=shas)
    dve_ops.OPS.append(op)
    dve_ops._SUB_OPCODE_FOR_NAME[name] = row
    return op


def _get_kan_ops():
    """Register (once) and return the two custom DVE ops:
    KAN_WIN:  a = relu(min(s0 - 11*x, 11*x - s1))  (= relu(2-|11x-(m+2)|)
              for s0 = m+4, s1 = m)
    KAN_CUBE: g = a^3 - 4*relu(a-1)^3              (= 6*N3(|11x-(m+2)|))
    """
    if "ops" in _DVE_OP_CACHE:
        return _DVE_OP_CACHE["ops"]
    from concourse.dve_spec import C0, C1, C2, One, Spec, Src0, minn, relu, sq

    m = Src0 * C2
    win_spec = Spec(
        body=relu(minn(C0 - m, m - C1)),
        reference=lambda in0, in1, s0, s1, imm2: np.maximum(
            np.minimum(s0 - in0 * imm2, in0 * imm2 - s1), 0.0
        ).astype(np.float32),
    )
    a = Src0
    b = relu(a - One)
    cube_spec = Spec(
        body=sq(a) * a + sq(b) * b * C2,
        reference=lambda in0, in1, s0, s1, imm2: (
            in0**3 + np.maximum(in0 - 1.0, 0.0) ** 3 * imm2
        ).astype(np.float32),
    )
    ops = (_register_dve_op("KAN_WIN_V1", win_spec),
           _register_dve_op("KAN_CUBE_V1", cube_spec))
    _DVE_OP_CACHE["ops"] = ops
    return ops


def _fold_coeff(coeff: np.ndarray):
    """coeff [COUT, CIN, KS, KS, NB] -> W_host [512, 576] in the matmul dtype.

    Channels fed to the matmul are 6*phi_m(t), so the folded weights are
    coeff/6 in layout W_host[m*64 + j, (kk*3+l)*64 + o]. Shipping them
    pre-cast (bf16) halves the head-of-kernel DMA bytes and avoids
    on-chip cast instructions.
    """
    w = (coeff.astype(np.float64).transpose(4, 1, 2, 3, 0) / 6.0).reshape(
        NS * CIN, TAPS * COUT)
    if MM_DTYPE_ENV == "bfloat16":
        import ml_dtypes

        return np.ascontiguousarray(w.astype(ml_dtypes.bfloat16))
    return np.ascontiguousarray(w, dtype=np.float32)


def _build_bass():
    import concourse.bacc as bacc
    import concourse.mybir as mybir
    import concourse.tile as tile

    f32 = mybir.dt.float32
    mm_dt = getattr(mybir.dt, MM_DTYPE_ENV)
    AF = mybir.ActivationFunctionType
    kan_win, kan_cube = _get_kan_ops()

    w_dt = mm_dt if mm_dt == mybir.dt.bfloat16 else f32

    nc = bacc.Bacc("TRN2", target_bir_lowering=False, debug=False,
                   num_devices=N_CORES)
    x_d = nc.dram_tensor("x", [B_SHARD, CIN, H, W], f32, kind="ExternalInput").ap()
    w_d = nc.dram_tensor("w", [NS * CIN, TAPS * COUT], w_dt,
                         kind="ExternalInput").ap()
    b_d = nc.dram_tensor("btbl", [128, 2 * NQ], f32, kind="ExternalInput").ap()
    out_d = nc.dram_tensor("out", [B_SHARD, COUT, HO, WO], f32,
                           kind="ExternalOutput").ap()

    with tile.TileContext(nc) as tc:
        from contextlib import ExitStack

        with ExitStack() as ctx:
            wpool = ctx.enter_context(tc.tile_pool(name="w", bufs=NQ))
            cpool = ctx.enter_context(tc.tile_pool(name="const", bufs=1))
            xpool = ctx.enter_context(tc.tile_pool(name="x", bufs=2))
            rpool = ctx.enter_context(tc.tile_pool(name="r", bufs=2))
            gpool = ctx.enter_context(tc.tile_pool(name="g", bufs=2))
            opool = ctx.enter_context(tc.tile_pool(name="o", bufs=2))
            ppool = ctx.enter_context(
                tc.tile_pool(name="ps", bufs=1, space="PSUM"))

            bt = cpool.tile([128, 2 * NQ], f32)
            nc.gpsimd.dma_start(bt[:], b_d[:])

            def emit_x(b):
                # x on the two HW-DGE queues (sync + scalar), one per
                # partition-half copy, so both transfer in parallel
                xts = []
                for h, (y0, nin) in enumerate(HALVES):
                    xt = xpool.tile([128, nin * W], f32, tag=f"xt{h}",
                                    name=f"xt{h}")
                    src = x_d[b, :, y0:y0 + nin, :]
                    xv = xt[:].rearrange("p (r c) -> p r c", c=W)
                    nc.sync.dma_start(xv[0:64], src)
                    nc.scalar.dma_start(xv[64:128], src)
                    xts.append(xt)
                return xts

            # weights ride the (otherwise idle) gpsimd SW-DGE queue in
            # parallel with the x loads
            xts0 = emit_x(0)
            wts = []
            for q in range(NQ):
                wt = wpool.tile([128, TAPS * COUT], w_dt, tag="wr", name="wr")
                nc.gpsimd.dma_start(wt[:], w_d[q * 128:(q + 1) * 128, :])
                wts.append(wt)

            for b in range(B_SHARD):
                # --- basis: q-outer so matmuls can start after chunk 0 ---
                xts = xts0 if b == 0 else emit_x(b)
                gts = {}  # (q, h) -> g tile [128, nrows*W] mm_dt
                for q in range(NQ):
                    for h, (y0, nin) in enumerate(HALVES):
                        npx = nin * W
                        a = rpool.tile([128, npx], f32, tag=f"a{h}",
                                       name=f"a{h}")
                        g = gpool.tile([128, npx], mm_dt, tag=f"g{q}{h}",
                                       name=f"g{q}{h}")
                        nc.vector._custom_dve(
                            kan_win, out=a[:], in0=xts[h][:],
                            s0=bt[:, 2 * q:2 * q + 1],
                            s1=bt[:, 2 * q + 1:2 * q + 2], imm2=11.0)
                        nc.vector._custom_dve(kan_cube, out=g[:], in0=a[:],
                                              imm2=-4.0)
                        gts[(q, h)] = g

                gvs = {k: g[:].rearrange("p (r c) -> p r c", c=W)
                       for k, g in gts.items()}

                # --- matmuls: tap-major, 8 interleaved psum chains ---
                pss = [ppool.tile([64, 8, WO], f32, tag=f"ps{g}",
                                  name=f"ps{g}")
                       for g, _ in GROUPS]
                step = 0
                for q in range(NQ):
                    for kk in range(KS):
                        for l in range(KS):
                            lhsT = wts[q][:, (kk * KS + l) * COUT:
                                          (kk * KS + l + 1) * COUT]
                            for g, nr in GROUPS:
                                h = g // 4
                                y0 = HALVES[h][0]
                                r0 = 8 * g + kk - y0
                                rhs = gvs[(q, h)][:, r0:r0 + nr, l:l + WO]
                                nc.tensor.matmul(
                                    pss[g][:, :nr, :], lhsT, rhs,
                                    start=(step == 0),
                                    stop=(step == N_STEPS - 1),
                                )
                            step += 1

                # --- drain: psum -> sbuf -> dram (split across S and V) ---
                for g, nr in GROUPS:
                    ot = opool.tile([64, 8, WO], f32, tag=f"ot{g % 4}",
                                    name=f"ot{g % 4}")
                    if g % 2 == 0:
                        nc.scalar.copy(ot[:, :nr, :], pss[g][:, :nr, :])
                    else:
                        nc.vector.tensor_copy(ot[:, :nr, :], pss[g][:, :nr, :])
                    nc.gpsimd.dma_start(
                        out_d[b, :, 8 * g:8 * g + nr, :], ot[:, :nr, :])

    nc.compile()
    return nc


def _maybe_install_profile_shim():
    """Allow trace=True/BASS_TRACE under axon even though this image lacks
    antenv.axon_hooks; degrade silently if anything is missing."""
    import types

    if "antenv.axon_hooks" in sys.modules:
        return
    try:
        from trn_agent_boot.trn_boot import _ntff_profile_via_ctypes

        hook = _ntff_profile_via_ctypes("/opt/axon/libaxon_pjrt.so")
        if hook is None:
            return
        mod = types.ModuleType("antenv.axon_hooks")
        mod.get_axon_ntff_profile_hook = lambda: hook
        mod.set_axon_ntff_profile_hook = lambda h: None
        sys.modules["antenv.axon_hooks"] = mod
        from concourse import bass_utils

        bass_utils.upload_artifacts = lambda tmpdir: f"local:{tmpdir}"
    except Exception:
        pass


_LAST_RESULTS = None


def kernel(x: np.ndarray, coeff: np.ndarray) -> np.ndarray:
    global _LAST_RESULTS
    from concourse import bass_utils

    _maybe_install_profile_shim()

    x = np.ascontiguousarray(np.asarray(x), dtype=np.float32)
    coeff = np.asarray(coeff)
    assert x.shape == (B_FULL, CIN, H, W), x.shape

    w_host = _fold_coeff(coeff)
    btbl = np.zeros((128, 2 * NQ), dtype=np.float32)
    for p in range(128):
        for q in range(NQ):
            m = 2 * q + (1 if p >= 64 else 0)
            btbl[p, 2 * q] = float(m + 4)      # s0: a = relu(min(s0-11x, 11x-s1))
            btbl[p, 2 * q + 1] = float(m)      # s1

    nc = _build_bass()

    in_maps = []
    for i in range(N_CORES):
        in_maps.append({
            "x": np.ascontiguousarray(x[i * B_SHARD:(i + 1) * B_SHARD]),
            "w": w_host,
            "btbl": btbl,
        })

    res = bass_utils.run_bass_kernel_spmd(
        nc, in_maps, core_ids=list(range(N_CORES)),
        trace=bool(os.environ.get("KAN_TRACE")),
    )
    _LAST_RESULTS = res

    out = np.concatenate([res.results[i]["out"] for i in range(N_CORES)], axis=0)
    return out.astype(np.float32, copy=False)


# revision 24
# speedup vs baseline: 1.1729x; 1.0597x over previous
"""Trainium2 Bass kernel for the ConvolutionalKAN problem.

Math: the KAN conv
    out[b,o,y,x] = sum_{j,kk,l,m} phi_m(11*inp[b,j,y+kk,x+l]) * coeff[o,j,kk,l,m]
with phi_m the degree-3 B-spline basis on uniform knots linspace(0,1,12).
Uniform knots -> phi_m(t) = N3(t-m) with N3 the cardinal cubic B-spline:
    6*N3 = a^3 - 4*b^3,  a = relu(2-u), b = relu(1-u) = relu(a-1), u = |t-(m+2)|
Weights fold to coeff/6 exactly, making this a VALID 3x3 conv over
64*8 = 512 input channels.

v2 design (vs the 217us baseline, which was elementwise-bound):
- The whole two-rail spline evaluation collapses to 2 Scalar ACTs
  (u = Abs(11x - (m+2)), a = Relu(2-u)) plus ONE custom DVE op
  KAN_CUBE: g = a^3 - 4*relu(a-1)^3 (exactly 8 ALU stages), registered
  at runtime in dve_ops.OPS. This removes ~350us of engine time.
- Basis tiles g are written in bf16 (halves SBUF + matmul streams at the
  same 1 col/cycle as f32r); weights cast to bf16.
- Matmuls run tap-major: for each (chunk q, tap) the stationary lhsT is
  loaded once and 8 interleaved psum chains (one per 8-row output group)
  stream against it, keeping the PE continuously busy so it ramps to the
  full 2.4 GHz p-state. Each group accumulates 36 matmuls (4 chunks x 9
  taps) into one PSUM bank; all 8 banks are in flight.
- Images are processed in row-halves (0..33 / 32..63) so the PE starts
  ~6us after kernel start instead of waiting for a full-image basis.

Sharding: data-parallel over batch, 2 images per core on 8 cores.
"""

import os
import sys

import numpy as np

for _p in ("/root/.axon_site/_ro/trn_rl_repo", "/opt/trn_rl_repo"):
    if os.path.isdir(_p) and _p not in sys.path:
        sys.path.append(_p)

B_FULL = 16
N_CORES = 8
B_SHARD = B_FULL // N_CORES
CIN = 64
COUT = 64
H = 64
W = 64
KS = 3
NB = 8
NS = 8
HO = H - KS + 1  # 62
WO = W - KS + 1  # 62
NQ = (CIN * NS) // 128  # 4 contraction tiles of 128
TAPS = KS * KS
N_STEPS = NQ * TAPS  # 36 accumulation steps per psum chain

# row strips: (first input row, n input rows); strip s feeds output row
# groups 2s and 2s+1 (group g reads input rows 8g .. 8g+nr+1)
STRIPS = [(0, 18), (16, 18), (32, 18), (48, 16)]
# output row groups: 8 groups of 8 rows (last has 6): group g = out rows
# 8g .. 8g+nr-1
GROUPS = [(g, 8 if g < 7 else 6) for g in range(8)]

MM_DTYPE_ENV = os.environ.get("KAN_MM_DTYPE", "bfloat16")

_DVE_OP_CACHE = {}


def _register_dve_op(name, spec):
    from concourse import dve_ops
    from concourse.dve_spec import lower
    from concourse.dve_uop import DveOpSpec
    from concourse.dve_spec import _has_src1

    existing = {op.name for op in dve_ops.OPS}
    if name in existing:
        return next(o for o in dve_ops.OPS if o.name == name)
    row = dve_ops._CUSTOM_DVE_ROW_BASE + len(dve_ops.OPS)
    shas = {}
    for ver in ("v3", "v4"):
        s = DveOpSpec(name=name, opcode=row, uops=lower(spec, ver=ver),
                      rd1_en=_has_src1(spec))
        shas[ver] = s.sha(ver)
    op = dve_ops.DveOp(name, spec, subdim=False, uops_sha=shas)
    dve_ops.OPS.append(op)
    dve_ops._SUB_OPCODE_FOR_NAME[name] = row
    return op


def _get_kan_ops():
    """Register (once) and return the two custom DVE ops:
    KAN_WIN:  a = relu(min(s0 - 11*x, 11*x - s1))  (= relu(2-|11x-(m+2)|)
              for s0 = m+4, s1 = m)
    KAN_CUBE: g = a^3 - 4*relu(a-1)^3              (= 6*N3(|11x-(m+2)|))
    """
    if "ops" in _DVE_OP_CACHE:
        return _DVE_OP_CACHE["ops"]
    from concourse.dve_spec import C0, C1, C2, One, Spec, Src0, minn, relu, sq

    m = Src0 * C2
    win_spec = Spec(
        body=relu(minn(C0 - m, m - C1)),
        reference=lambda in0, in1, s0, s1, imm2: np.maximum(
            np.minimum(s0 - in0 * imm2, in0 * imm2 - s1), 0.0
        ).astype(np.float32),
    )
    a = Src0
    b = relu(a - One)
    cube_spec = Spec(
        body=sq(a) * a + sq(b) * b * C2,
        reference=lambda in0, in1, s0, s1, imm2: (
            in0**3 + np.maximum(in0 - 1.0, 0.0) ** 3 * imm2
        ).astype(np.float32),
    )
    ops = (_register_dve_op("KAN_WIN_V1", win_spec),
           _register_dve_op("KAN_CUBE_V1", cube_spec))
    _DVE_OP_CACHE["ops"] = ops
    return ops


def _fold_coeff(coeff: np.ndarray):
    """coeff [COUT, CIN, KS, KS, NB] -> W_host [512, 576] in the matmul dtype.

    Channels fed to the matmul are 6*phi_m(t), so the folded weights are
    coeff/6 in layout W_host[m*64 + j, (kk*3+l)*64 + o]. Shipping them
    pre-cast (bf16) halves the head-of-kernel DMA bytes and avoids
    on-chip cast instructions.
    """
    w = (coeff.astype(np.float64).transpose(4, 1, 2, 3, 0) / 6.0).reshape(
        NS * CIN, TAPS * COUT)
    if MM_DTYPE_ENV == "bfloat16":
        import ml_dtypes

        return np.ascontiguousarray(w.astype(ml_dtypes.bfloat16))
    return np.ascontiguousarray(w, dtype=np.float32)


def _build_bass():
    import concourse.bacc as bacc
    import concourse.mybir as mybir
    import concourse.tile as tile

    f32 = mybir.dt.float32
    mm_dt = getattr(mybir.dt, MM_DTYPE_ENV)
    AF = mybir.ActivationFunctionType
    kan_win, kan_cube = _get_kan_ops()

    w_dt = mm_dt if mm_dt == mybir.dt.bfloat16 else f32

    nc = bacc.Bacc("TRN2", target_bir_lowering=False, debug=False,
                   num_devices=N_CORES)
    x_d = nc.dram_tensor("x", [B_SHARD, CIN, H, W], f32, kind="ExternalInput").ap()
    w_d = nc.dram_tensor("w", [NS * CIN, TAPS * COUT], w_dt,
                         kind="ExternalInput").ap()
    b_d = nc.dram_tensor("btbl", [128, 2 * NQ], f32, kind="ExternalInput").ap()
    out_d = nc.dram_tensor("out", [B_SHARD, COUT, HO, WO], f32,
                           kind="ExternalOutput").ap()

    with tile.TileContext(nc) as tc:
        from contextlib import ExitStack

        with ExitStack() as ctx:
            wpool = ctx.enter_context(tc.tile_pool(name="w", bufs=NQ))
            cpool = ctx.enter_context(tc.tile_pool(name="const", bufs=1))
            xpool = ctx.enter_context(tc.tile_pool(name="x", bufs=2))
            rpool = ctx.enter_context(tc.tile_pool(name="r", bufs=2))
            gpool = ctx.enter_context(tc.tile_pool(name="g", bufs=2))
            opool = ctx.enter_context(tc.tile_pool(name="o", bufs=2))
            ppool = ctx.enter_context(
                tc.tile_pool(name="ps", bufs=1, space="PSUM"))

            bt = cpool.tile([128, 2 * NQ], f32)
            nc.gpsimd.dma_start(bt[:], b_d[:])

            def emit_x(b):
                # x on the two HW-DGE queues (sync + scalar), one per
                # partition-half copy, so both transfer in parallel
                xts = []
                for s, (y0, nin) in enumerate(STRIPS):
                    xt = xpool.tile([128, nin * W], f32, tag=f"xt{s}",
                                    name=f"xt{s}")
                    src = x_d[b, :, y0:y0 + nin, :]
                    xv = xt[:].rearrange("p (r c) -> p r c", c=W)
                    nc.sync.dma_start(xv[0:64], src)
                    nc.scalar.dma_start(xv[64:128], src)
                    xts.append(xt)
                return xts

            # weights ride the (otherwise idle) gpsimd SW-DGE queue in
            # parallel with the x loads
            xts0 = emit_x(0)
            wts = []
            for q in range(NQ):
                wt = wpool.tile([128, TAPS * COUT], w_dt, tag="wr", name="wr")
                nc.gpsimd.dma_start(wt[:], w_d[q * 128:(q + 1) * 128, :])
                wts.append(wt)

            for b in range(B_SHARD):
                # --- basis: q-outer, strip-inner, so matmuls start after
                # (q0, strip0) ---
                xts = xts0 if b == 0 else emit_x(b)
                gts = {}  # (q, s) -> g tile [128, nrows*W] mm_dt
                for q in range(NQ):
                    for s, (y0, nin) in enumerate(STRIPS):
                        npx = nin * W
                        a = rpool.tile([128, npx], f32, tag=f"a{s}",
                                       name=f"a{s}")
                        g = gpool.tile([128, npx], mm_dt, tag=f"g{q}{s}",
                                       name=f"g{q}{s}")
                        nc.vector._custom_dve(
                            kan_win, out=a[:], in0=xts[s][:],
                            s0=bt[:, 2 * q:2 * q + 1],
                            s1=bt[:, 2 * q + 1:2 * q + 2], imm2=11.0)
                        nc.vector._custom_dve(kan_cube, out=g[:], in0=a[:],
                                              imm2=-4.0)
                        gts[(q, s)] = g

                gvs = {k: g[:].rearrange("p (r c) -> p r c", c=W)
                       for k, g in gts.items()}

                # --- matmuls: 8 interleaved psum chains; group-within-q
                # order so chain (q0, g0) only needs strip 0 ---
                pss = [ppool.tile([64, 8, WO], f32, tag=f"ps{g}",
                                  name=f"ps{g}")
                       for g, _ in GROUPS]
                for q in range(NQ):
                    for g, nr in GROUPS:
                        s = g // 2
                        y0 = STRIPS[s][0]
                        for kk in range(KS):
                            for l in range(KS):
                                lhsT = wts[q][:, (kk * KS + l) * COUT:
                                              (kk * KS + l + 1) * COUT]
                                r0 = 8 * g + kk - y0
                                rhs = gvs[(q, s)][:, r0:r0 + nr, l:l + WO]
                                nc.tensor.matmul(
                                    pss[g][:, :nr, :], lhsT, rhs,
                                    start=(q == 0 and kk == 0 and l == 0),
                                    stop=(q == NQ - 1 and kk == KS - 1
                                          and l == KS - 1),
                                )

                # --- drain: psum -> sbuf -> dram (split across S and V) ---
                for g, nr in GROUPS:
                    ot = opool.tile([64, 8, WO], f32, tag=f"ot{g % 4}",
                                    name=f"ot{g % 4}")
                    if g % 2 == 0:
                        nc.scalar.copy(ot[:, :nr, :], pss[g][:, :nr, :])
                        nc.scalar.dma_start(
                            out_d[b, :, 8 * g:8 * g + nr, :], ot[:, :nr, :])
                    else:
                        nc.vector.tensor_copy(ot[:, :nr, :], pss[g][:, :nr, :])
                        nc.sync.dma_start(
                            out_d[b, :, 8 * g:8 * g + nr, :], ot[:, :nr, :])

    nc.compile()
    return nc


def _maybe_install_profile_shim():
    """Allow trace=True/BASS_TRACE under axon even though this image lacks
    antenv.axon_hooks; degrade silently if anything is missing."""
    import types

    if "antenv.axon_hooks" in sys.modules:
        return
    try:
        from trn_agent_boot.trn_boot import _ntff_profile_via_ctypes

        hook = _ntff_profile_via_ctypes("/opt/axon/libaxon_pjrt.so")
        if hook is None:
            return
        mod = types.ModuleType("antenv.axon_hooks")
        mod.get_axon_ntff_profile_hook = lambda: hook
        mod.set_axon_ntff_profile_hook = lambda h: None
        sys.modules["antenv.axon_hooks"] = mod
        from concourse import bass_utils

        bass_utils.upload_artifacts = lambda tmpdir: f"local:{tmpdir}"
    except Exception:
        pass


_LAST_RESULTS = None


def kernel(x: np.ndarray, coeff: np.ndarray) -> np.ndarray:
    global _LAST_RESULTS
    from concourse import bass_utils

    _maybe_install_profile_shim()

    x = np.ascontiguousarray(np.asarray(x), dtype=np.float32)
    coeff = np.asarray(coeff)
    assert x.shape == (B_FULL, CIN, H, W), x.shape

    w_host = _fold_coeff(coeff)
    btbl = np.zeros((128, 2 * NQ), dtype=np.float32)
    for p in range(128):
        for q in range(NQ):
            m = 2 * q + (1 if p >= 64 else 0)
            btbl[p, 2 * q] = float(m + 4)      # s0: a = relu(min(s0-11x, 11x-s1))
            btbl[p, 2 * q + 1] = float(m)      # s1

    nc = _build_bass()

    in_maps = []
    for i in range(N_CORES):
        in_maps.append({
            "x": np.ascontiguousarray(x[i * B_SHARD:(i + 1) * B_SHARD]),
            "w": w_host,
            "btbl": btbl,
        })

    res = bass_utils.run_bass_kernel_spmd(
        nc, in_maps, core_ids=list(range(N_CORES)),
        trace=bool(os.environ.get("KAN_TRACE")),
    )
    _LAST_RESULTS = res

    out = np.concatenate([res.results[i]["out"] for i in range(N_CORES)], axis=0)
    return out.astype(np.float32, copy=False)
